# revision 1
# baseline (speedup 1.0000x reference)
"""GAT (graph attention) kernel for Trainium2, sharded across 8 NeuronCores.

Math per head h:
    e   = leakyrelu(src_i + tgt_j)            (slope 0.2)
    att = softmax(where(mask, e, -9e16))
    out = att_E @ ht_e + att_N @ ht_n, then mean over heads.

Identity: with s = src_i + tgt_j,
    exp(leakyrelu(s)) = e^{0.2 src_i} * max(P'_i * QT_j, T_j)
    P'_i = e^{0.8 src_i},  QT_j = e^{tgt_j},  T_j = e^{0.2 tgt_j}
The per-row factor e^{0.2 src_i} cancels in the softmax, so the
unnormalised attention is u[j,i] = mask[j,i] * max(P'_i*QT_j, T_j):
one tensor_scalar (mult+max, DVE 4x mode) plus one mask tensor_mul
(2x mode) per element.  The softmax denominator comes from a column of
H's appended to V, which also folds in the 1/H head-mean.

Engine balance (v2 cost model rates):
  - tensor_scalar score pass split DVE/Pool ~half-half per group
    (POOL_TS_CHUNKS); the mask tensor_mul stays on DVE (Pool's 0.42
    multiply efficiency makes it a poor fit there).
  - Act: PSUM->SBUF copies (ht/V tiles, P' broadcast rows), exps.
  - PE: score matmuls, ht matmuls, and the attention @ V accumulation
    in the i-stationary orientation: lhsT = masked-attention chunk
    [j, i-half], rhs = [V | H] columns, so S^T[i, hd|den] lands in
    PSUM directly (no transposes) and normalize is one reciprocal +
    one fused scale-accumulate per row chunk, deferred into the next
    head's compute to hide the PE->DVE semaphore latency.

Sharding: destination rows (N dim) split across 8 cores, 256 rows
each (row-parallel attention); embeddings/weights replicated.  Host
does casts / transposes / slicing and folds W@a into the small
per-head score vectors wv (weight preprocessing); small constant
tensors ship packed in one "combo" DMA to dodge per-DMA descriptor
serialization.
"""

from contextlib import ExitStack

import numpy as np

import concourse.bass as bass
import concourse.bacc as bacc
import concourse.mybir as mybir
import concourse.tile as tile
from concourse.bass_utils import run_bass_kernel_spmd

N, E, F_IN, H, D = 2048, 4096, 256, 8, 64
NCORES = 8
R = N // NCORES          # 256 destination rows per core
RC = R // 128            # 2 row chunks of 128
NCH = N // 128           # 16 node j-chunks
ECH = E // 128           # 32 edge j-chunks
GP = (16, 16)            # j-chunks per mask group, per part
F16 = mybir.dt.float16
F32 = mybir.dt.float32

# (part, head, group) -> mask tensor_mul on Pool instead of DVE (unused
# in the tuned config; Pool's Multiply efficiency is poor).
POOL_GROUPS = set()

WORK_BUFS = 5
POOL_TS_CHUNKS = (7, 8)
PB_ON_ACT = True
FLIP = True
POOL_TS_FN = None
INPLACE_MASK = False

_PROGRAM = None


def _build_program() -> bass.Bass:
    nc = bacc.Bacc("TRN2", target_bir_lowering=False, debug=False)

    maskN_d = nc.dram_tensor("maskN", [N, R], F16, kind="ExternalInput")
    maskE_d = nc.dram_tensor("maskE", [E, R], F16, kind="ExternalInput")
    nodesT_d = nc.dram_tensor("nodesT", [F_IN, N], F16, kind="ExternalInput")
    edgesT_d = nc.dram_tensor("edgesT", [F_IN, E], F16, kind="ExternalInput")
    combo_d = nc.dram_tensor("combo", [128, 704], F16, kind="ExternalInput")
    WN_d = nc.dram_tensor("WN16", [F_IN, H * D], F16, kind="ExternalInput")
    WE_d = nc.dram_tensor("WE16", [F_IN, H * D], F16, kind="ExternalInput")
    sel16_d = nc.dram_tensor("sel16c", [2 * H, 2 * H * 128], F16, kind="ExternalInput")
    out_d = nc.dram_tensor("out", [R, D], F32, kind="ExternalOutput")

    Copy = mybir.ActivationFunctionType.Copy
    Exp = mybir.ActivationFunctionType.Exp
    MULT = mybir.AluOpType.mult
    MAX = mybir.AluOpType.max
    ADD = mybir.AluOpType.add

    with tile.TileContext(nc) as tc, ExitStack() as ctx:
        singles = ctx.enter_context(tc.tile_pool(name="singles", bufs=1))
        work = ctx.enter_context(tc.tile_pool(name="work", bufs=WORK_BUFS))
        small = ctx.enter_context(tc.tile_pool(name="small", bufs=6))
        psum_ht = ctx.enter_context(tc.tile_pool(name="psum_ht", bufs=2, space="PSUM"))
        psum_acc = ctx.enter_context(tc.tile_pool(name="psum_acc", bufs=2, space="PSUM"))
        psum_misc = ctx.enter_context(tc.tile_pool(name="psum_misc", bufs=2, space="PSUM"))

        # ---- persistent SBUF arrays -------------------------------------
        maskN = singles.tile([128, NCH, R], F16, tag="maskN")
        maskE = singles.tile([128, ECH, R], F16, tag="maskE")
        nodesT = singles.tile([128, 2, N], F16, tag="nodesT")
        edgesT = singles.tile([128, 2, E], F16, tag="edgesT")
        combo = singles.tile([128, 704], F16, tag="combo")
        WN = singles.tile([128, 2, H * D], F16, tag="WN")
        WE = singles.tile([128, 2, H * D], F16, tag="WE")
        # per-j exponential vectors (j-chunk partition layout)
        e10nA = singles.tile([128, NCH // 2, 3 * H], F32, tag="e10nA")
        e10nB = singles.tile([128, NCH // 2, 3 * H], F32, tag="e10nB")
        e2nA = singles.tile([128, NCH // 2, 3 * H], F32, tag="e2nA")
        e2nB = singles.tile([128, NCH // 2, 3 * H], F32, tag="e2nB")
        e10e = singles.tile([128, ECH, H], F32, tag="e10e")
        e2e = singles.tile([128, ECH, H], F32, tag="e2e")
        # V tiles: [ht | 1] per (j-chunk, head); 66-wide for 4B alignment
        VT = singles.tile([128, NCH + ECH, H, 66], F16, tag="VT")
        # P' broadcast tiles per (head, part) - separate tiles so a
        # reader only depends on its own (head, part) broadcast
        Pbt = [singles.tile([128, R], F16, tag=f"Pb{u}", name=f"Pb{u}")
               for u in range(2 * H)]
        ptsb = singles.tile([2 * H, RC, 128], F16, tag="ptsb")
        sel16 = singles.tile([2 * H, 2 * H * 128], F16, tag="sel16")
        acc = singles.tile([128, RC, D], F32, tag="acc")
        wvN = combo[:, 0:48].rearrange("p (c m) -> p c m", c=2)
        ownT = combo[:, 48:560].rearrange("p (c m) -> p c m", c=2)
        identF16 = combo[:, 560:688]
        wvE = combo[:, 688:704].rearrange("p (c m) -> p c m", c=2)

        dum = singles.tile([1, 2], F32, tag="dum")
        nc.vector.memset(dum[0:1, 0:1], 1.0)
        nc.scalar.activation(dum[0:1, 1:2], dum[0:1, 0:1], Exp)
        nc.gpsimd.memset(VT[:, :, :, 64:66], 0.0)
        nc.gpsimd.memset(VT[:, :, :, 64:65], float(H))
        nc.gpsimd.memset(acc, 0.0)

        # ---- input DMAs (critical-path tensors first) --------------------
        nodesT_r = nodesT_d.rearrange("(c p) n -> p c n", p=128)
        edgesT_r = edgesT_d.rearrange("(c p) n -> p c n", p=128)
        maskN_r = maskN_d.rearrange("(t p) i -> p t i", p=128)
        maskE_r = maskE_d.rearrange("(t p) i -> p t i", p=128)
        nc.sync.dma_start(out=combo, in_=combo_d[:, :])
        nc.sync.dma_start(out=sel16, in_=sel16_d[:, :])
        nc.sync.dma_start(out=maskN[:, 0:NCH // 2, :], in_=maskN_r[:, 0:NCH // 2, :])
        nc.sync.dma_start(out=nodesT[:, :, 0:N // 2], in_=nodesT_r[:, :, 0:N // 2])
        nc.sync.dma_start(out=nodesT[:, :, N // 2:], in_=nodesT_r[:, :, N // 2:])
        nc.sync.dma_start(out=maskN[:, NCH // 2:, :], in_=maskN_r[:, NCH // 2:, :])
        nc.sync.dma_start(out=WN, in_=WN_d.rearrange("(c p) m -> p c m", p=128))
        nc.sync.dma_start(out=edgesT, in_=edgesT_r)
        nc.sync.dma_start(out=WE, in_=WE_d.rearrange("(c p) m -> p c m", p=128))
        nc.sync.dma_start(out=maskE, in_=maskE_r)

        # ---- own-row P' = e^{0.8 src} -> free-dim broadcast tiles --------
        pso = psum_misc.tile([128, 2, 3 * H], F32, tag="pm")
        for ch in range(RC):
            for kc in range(2):
                nc.tensor.matmul(pso[:, ch, :],
                                 ownT[:, kc, ch * 128:(ch + 1) * 128],
                                 wvN[:, kc, :], start=(kc == 0), stop=(kc == 1))
        e8own = small.tile([128, RC, 3 * H], F16, tag="e8own")
        nc.scalar.activation(e8own[:, :, :], pso[:, :, :], Exp, scale=0.8)
        # pick the 16 src columns (u = 2h+part <- col 3h+2*part), transpose
        # to [16, i] rows, then partition-broadcast each row on Pool.
        e8cols = small.tile([128, RC, 2 * H], F16, tag="e8cols")
        for ch in range(RC):
            e8sl = e8own[:, ch, :]
            cols = bass.AP(tensor=e8sl.tensor, offset=e8sl.offset,
                           ap=[e8sl.ap[0], [3, H], [2, 2]])
            nc.vector.tensor_copy(e8cols[:, ch, :], cols)
            pt = psum_misc.tile([2 * H, 128], F16, tag="pm")
            nc.tensor.transpose(pt[:, :], e8cols[:, ch, :], identF16[:, :])
            nc.scalar.activation(ptsb[:, ch, :], pt[:, :], Copy)
        def emit_pb(u):
            h, part = u // 2, u % 2
            pb = psum_misc.tile([128, R], F32, tag="pm")
            nc.tensor.matmul(pb[:, :], sel16[:, u * 128:(u + 1) * 128],
                             ptsb[:, :, :].rearrange("u c p -> u (c p)"))
            if PB_ON_ACT and u >= 2:
                nc.scalar.activation(Pbt[u][:, :], pb[:, :], Copy)
            else:
                nc.vector.tensor_copy(Pbt[u][:, :], pb[:, :])

        # ---- src/tgt scores -> per-j exponentials ------------------------
        psn = psum_misc.tile([128, NCH, 3 * H], F32, tag="pm")

        def emit_psn_half(half, e10h, e2h):
            for ch in range(half * NCH // 2, (half + 1) * NCH // 2):
                for kc in range(2):
                    nc.tensor.matmul(psn[:, ch, :],
                                     nodesT[:, kc, ch * 128:(ch + 1) * 128],
                                     wvN[:, kc, :], start=(kc == 0), stop=(kc == 1))
            h0, h1 = half * NCH // 2, (half + 1) * NCH // 2
            nc.scalar.activation(e10h[:, :, :], psn[:, h0:h1, :], Exp, scale=1.0)
            nc.scalar.activation(e2h[:, :, :], psn[:, h0:h1, :], Exp, scale=0.2)

        for u in range(2):
            emit_pb(u)
        emit_psn_half(0, e10nA, e2nA)
        emit_psn_half(1, e10nB, e2nB)
        for u in range(2, 2 * H):
            emit_pb(u)

        # ---- ht = emb @ W, stored as [ht | 1] f16 V tiles ----------------
        def emit_ht(ch):
            ph = psum_ht.tile([128, H * D], F32, tag="ph")
            for kc in range(2):
                if ch < NCH:
                    lhsT = nodesT[:, kc, ch * 128:(ch + 1) * 128]
                    rhs = WN[:, kc, :]
                else:
                    lhsT = edgesT[:, kc, (ch - NCH) * 128:(ch - NCH + 1) * 128]
                    rhs = WE[:, kc, :]
                nc.tensor.matmul(ph[:, :], lhsT, rhs, start=(kc == 0), stop=(kc == 1))
            nc.scalar.activation(
                VT[:, ch, :, 0:64],
                ph[:, :].rearrange("p (h d) -> p h d", h=H),
                Copy,
            )

        for ch in range(NCH):
            emit_ht(ch)

        def emit_edges_prep():
            pse = psum_misc.tile([128, ECH, H], F32, tag="pm")
            for ch in range(ECH):
                for kc in range(2):
                    nc.tensor.matmul(pse[:, ch, :],
                                     edgesT[:, kc, ch * 128:(ch + 1) * 128],
                                     wvE[:, kc, :], start=(kc == 0), stop=(kc == 1))
            nc.scalar.activation(e10e[:, :, :], pse[:, :, :], Exp, scale=1.0)
            nc.scalar.activation(e2e[:, :, :], pse[:, :, :], Exp, scale=0.2)
            for ch in range(NCH, NCH + ECH):
                emit_ht(ch)

        # ---- main loop: u = mask * max(P'*QT, T); S += u^T-contract V ----
        norm_pending = []
        for part in range(2):
            njt = NCH if part == 0 else ECH
            G = GP[part]
            for h in range(H):
                if part == 0 and h == 3:
                    emit_edges_prep()
                Sp0 = psum_acc.tile([128, 65], F32, tag="Sacc0")
                Sp1 = psum_acc.tile([128, 65], F32, tag="Sacc1")
                Sps = [Sp0, Sp1]
                for grp, jt0 in enumerate(range(0, njt, G)):
                    dt_ = work.tile([128, G, R], F16, tag="Dt")
                    for g in range(G):
                        jt = jt0 + g
                        if part == 0:
                            e10h = e10nA if jt < NCH // 2 else e10nB
                            e2h = e2nA if jt < NCH // 2 else e2nB
                            jh = jt % (NCH // 2)
                            q10 = e10h[:, jh, 3 * h + 1:3 * h + 2]
                            q2 = e2h[:, jh, 3 * h + 1:3 * h + 2]
                        else:
                            q10 = e10e[:, jt, h:h + 1]
                            q2 = e2e[:, jt, h:h + 1]
                        npool = (POOL_TS_FN(part, h) if POOL_TS_FN else
                                 POOL_TS_CHUNKS[part])
                        ts_eng = nc.gpsimd if g < npool else nc.vector
                        ts_eng.tensor_scalar(
                            out=dt_[:, g, :], in0=Pbt[2 * h + part][:, :],
                            scalar1=q10, scalar2=q2,
                            op0=MULT, op1=MAX,
                        )
                    msk = maskN if part == 0 else maskE
                    if INPLACE_MASK:
                        nc.vector.tensor_tensor(
                            out=dt_[:, :, :], in0=dt_[:, :, :],
                            in1=msk[:, jt0:jt0 + G, :], op=MULT)
                        mt = dt_
                    else:
                        ut = work.tile([128, G, R], F16, tag="ut")
                        eng = (nc.gpsimd if (part, h, grp) in POOL_GROUPS
                               else nc.vector)
                        eng.tensor_tensor(out=ut[:, :, :], in0=dt_[:, :, :],
                                          in1=msk[:, jt0:jt0 + G, :], op=MULT)
                        mt = ut
                    for g in range(G):
                        jt = jt0 + g
                        vch = jt if part == 0 else NCH + jt
                        for ih in range(RC):
                            nc.tensor.matmul(
                                Sps[ih][:, :],
                                mt[:, g, ih * 128:(ih + 1) * 128],
                                VT[:, vch, h, 0:65],
                                start=(jt == 0), stop=(jt == njt - 1),
                            )
                # ---- normalize (deferred into the next head's compute) --
                Ssb = small.tile([128, RC, 65], F32, tag="Ssb")
                for ch in range(RC):
                    nc.scalar.activation(Ssb[:, ch, :], Sps[ch][:, :], Copy)

                def make_norm(Ssb_):
                    def emit_norm():
                        for ch in range(RC):
                            rec = small.tile([128, 1], F32, tag="rec")
                            nc.vector.reciprocal(rec[:, :], Ssb_[:, ch, 64:65])
                            nc.vector.scalar_tensor_tensor(
                                out=acc[:, ch, :], in0=Ssb_[:, ch, 0:64],
                                scalar=rec[:, :], in1=acc[:, ch, :],
                                op0=MULT, op1=ADD,
                            )
                    return emit_norm
                norm_pending.append(make_norm(Ssb))
                if len(norm_pending) > 1:
                    norm_pending.pop(0)()

        for fn_ in norm_pending:
            fn_()
        norm_pending = []
        nc.sync.dma_start(out=out_d.rearrange("(c p) d -> p c d", p=128), in_=acc)

    return nc


def _get_program() -> bass.Bass:
    global _PROGRAM
    if _PROGRAM is None:
        nc = _build_program()
        nc.finalize()
        _PROGRAM = nc
    return _PROGRAM


def _prepare_in_maps(inputs) -> list:
    nodes = np.ascontiguousarray(np.asarray(inputs["nodes_embeddings"], np.float32))
    edges = np.ascontiguousarray(np.asarray(inputs["edges_embeddings"], np.float32))
    WN = np.asarray(inputs["WN"], np.float32)
    WE = np.asarray(inputs["WE"], np.float32)
    aN = np.asarray(inputs["aN"], np.float32)
    aE = np.asarray(inputs["aE"], np.float32)
    mat_nodes = np.asarray(inputs["mat_nodes"])
    mat_edges = np.asarray(inputs["mat_edges"])

    f16 = np.float16
    nodesT16 = np.ascontiguousarray(nodes.T.astype(f16))
    edgesT16 = np.ascontiguousarray(edges.T.astype(f16))
    WN16 = WN.astype(f16)
    WE16 = WE.astype(f16)
    # wv[k, c] = sum_d W[k, h*D+d] * a[h, d, col]; per-head col order for
    # wvN16: [srcN, tgtN, srcE]; wvE16: [tgtE]
    WNr = WN.reshape(F_IN, H, D)
    WEr = WE.reshape(F_IN, H, D)
    wvN16 = np.empty((F_IN, 3 * H), f16)
    wvE16 = np.empty((F_IN, H), f16)
    for h in range(H):
        wvN16[:, 3 * h] = (WNr[:, h, :] @ aN[h, :D, 0]).astype(f16)
        wvN16[:, 3 * h + 1] = (WNr[:, h, :] @ aN[h, D:, 0]).astype(f16)
        wvN16[:, 3 * h + 2] = (WNr[:, h, :] @ aE[h, :D, 0]).astype(f16)
        wvE16[:, h] = (WEr[:, h, :] @ aE[h, D:, 0]).astype(f16)
    sel16c = np.zeros((2 * H, 2 * H * 128), f16)
    for u in range(2 * H):
        sel16c[u, u * 128:(u + 1) * 128] = 1.0
    idf16 = np.eye(128, dtype=f16)
    maskN_T = np.ascontiguousarray(mat_nodes.astype(f16).T)  # [j, i]
    maskE_T = np.ascontiguousarray(mat_edges.astype(f16).T)

    in_maps = []
    for c in range(NCORES):
        sl = slice(c * R, (c + 1) * R)
        in_maps.append({
            "maskN": np.ascontiguousarray(maskN_T[:, sl]),
            "maskE": np.ascontiguousarray(maskE_T[:, sl]),
            "nodesT": nodesT16,
            "edgesT": edgesT16,
            "combo": np.concatenate([
                wvN16.reshape(2, 128, 3 * H).transpose(1, 0, 2).reshape(128, 6 * H),
                nodesT16[:, sl].reshape(2, 128, R).transpose(1, 0, 2).reshape(128, 2 * R),
                idf16,
                wvE16.reshape(2, 128, H).transpose(1, 0, 2).reshape(128, 2 * H),
            ], axis=1),
            "WN16": WN16,
            "WE16": WE16,
            "sel16c": sel16c,
        })
    return in_maps


def kernel(**inputs) -> np.ndarray:
    in_maps = _prepare_in_maps(inputs)
    nc = _get_program()
    res = run_bass_kernel_spmd(nc, in_maps, core_ids=list(range(NCORES)))
    return np.concatenate([res.results[c]["out"] for c in range(NCORES)], axis=0)

